# revision 34
# baseline (speedup 1.0000x reference)
"""CTC batch loss kernel for Trainium2 (8 NeuronCores, batch-parallel).

Math: reference computes logp = log_softmax(log(y+eps)) = log(y+eps) - log(rowsum),
then a log-space forward DP over the extended label sequence (S = 2L+1 = 129).
We run the DP in probability space with periodic renormalization.

v5 structure:
  - MERGED chains: the backward DP is recast as a forward-structured DP via
    phi_t = beta_t * emit_t, state-mirrored (psi[m] = phi[S-1-m]).  psi obeys
    the EXACT same recurrence as alpha:
        e_new[i] = e[i] + o[i-1]
        o_new[j] = (e[j]+o[j])*ul[j] + o[j-1]*su[j]
    with time-reversed, label-mirrored emissions.  Both chains live in ONE
    set of tiles on 64 partitions (rows 0:32 fwd samples, rows 32:64 bwd).
  - Each DP step is 4 DVE instructions ordered [I1, x2, x1, I3] so that x2
    (independent of I1) sits between I1 and its consumer x1: the tile
    framework completion-chains distance-1 dependent DVE ops (~255ns each),
    while ops with an instruction between them flow at the ~110-140ns issue
    rate.  This cuts the step from ~840ns (3 fully-chained ops) to ~700ns.
    o is stored ONCE; [(0,o)|(o,0)] is an overlapping 2-segment AP
    (segment stride = 1 element) over it.
  - fwd runs t=1..127 (alpha_127), bwd runs t=254..128 (psi_128), then a
    "fake" fwd step with emissions [skip | 1] yields the pre-emission
    propagation a~, and  loglik~ = sum_i a~e[i]*psiE[64-i] + sum_j
    a~o[j]*psio[63-j]  (reversed dot via negative-stride APs after an
    SBUF->SBUF DMA brings psi rows down to partitions 0:32).
  - Rowsum path: raw y (NOT divided by ub) streamed as fp8 e4m3; PE matmuls
    in DoubleRow mode (256-deep contraction) with 2 samples per lhsT
    -> PSUM [128,{t x 2samples},4pairs]; Ln on ACT (scale 2^-16); per-sample
    t-reduction via two f32 matmuls per quarter into PR [32,1] PSUM.
    The host supplies hc[b] = sum_t log(ub_t) as a tiny f32 input.
  - Renorm every 32 steps (both chains in one reduce/scale); Sqrt+Ln of the
    renorm factors deferred to the tail (2 ACT table switches total).
  - loss = PR - hc - sum_r log c_r - log(loglik~) + 3840*ln2
"""

import math
import sys
from contextlib import ExitStack

import numpy as np

sys.path.insert(0, "/opt/trn_rl_repo")
sys.path.insert(0, "/root/.axon_site/_ro/trn_rl_repo")

import ml_dtypes  # noqa: E402

B, T, C, L = 256, 256, 1024, 64
NCORES = 8
BS = B // NCORES  # 32 samples per core
NQ = 4
TQW = T // NQ  # 64
NPAIR = BS // 2  # 16 (pair p = samples (p, p+16))
NGRP = 4  # pair-groups per quarter DMA
KCH = 8  # 128-deep contraction chunks
BLANK = C - 1
EPS = 1e-7
LNSC = float(2.0 ** -16)
RENORMS = (31, 63, 95, 127)
NN = len(RENORMS)
EMCOLS = 129  # col 0 = init, 1..127 = steps, 128 = fake (skip|1)


# ---------------------------------------------------------------- host prep

def host_prep(y_pred: np.ndarray, y_true: np.ndarray):
    """Returns per-core arrays:
       yt [NCORES, NQ, 128, NPAIR, KCH, 2, TQW]  fp8 e4m3 (raw y)
       em [NCORES, 64, EMCOLS, 128]              bf16
       hc [NCORES, BS, 1]                        f32 (sum_t log ub)
    """
    y = np.asarray(y_pred, dtype=np.float32)
    ub = y[:, :, BLANK] + EPS  # [B, T]
    hc = np.log(ub).sum(axis=1).astype(np.float32)  # [B]

    # --- rowsum stream: raw y, fp8 -----------------------------------------
    # channel = k*128 + c ; sample b = s*16 + p (within core)
    yt = y.reshape(NCORES, 2, NPAIR, NQ, TQW, KCH, 128)
    yt = yt.transpose(0, 3, 6, 2, 5, 1, 4)  # -> [nc, q, c, p, k, s, t]
    yt = np.ascontiguousarray(yt).astype(ml_dtypes.float8_e4m3fn)

    # --- DP emissions ------------------------------------------------------
    lab = np.asarray(y_true).astype(np.int64)
    bidx = np.arange(B)[:, None, None]
    tidx = np.arange(T)[None, :, None]
    ul = y[bidx, tidx, lab[:, None, :]] / ub[:, :, None]  # [B, T, L]
    skip = np.zeros((B, L), dtype=np.float32)
    skip[:, 1:] = (lab[:, 1:] != lab[:, :-1]).astype(np.float32)
    su = ul * skip[:, None, :]
    # bwd (mirrored): ul'[j'] = ul[63-j'], su'[j'] = skip[64-j']*ul'[j']
    ulp = ul[:, :, ::-1]
    skipp = np.zeros((B, L), dtype=np.float32)
    skipp[:, 1:] = skip[:, 1:][:, ::-1]
    sup = ulp * skipp[:, None, :]

    em = np.zeros((B, 2, EMCOLS, 128), dtype=np.float32)
    # fwd half (row group 0): col c = t=c ; col 128 = [skip | 1]
    em[:, 0, :128, 0:L] = su.transpose(0, 1, 2)[:, :128][:, :, :]
    em[:, 0, :128, L:] = ul[:, :128]
    em[:, 0, 128, 0:L] = skip
    em[:, 0, 128, L:] = 1.0
    # bwd half (row group 1): col c = t=255-c ; col 128 = 0
    em[:, 1, :128, 0:L] = sup[:, 255:127:-1]
    em[:, 1, :128, L:] = ulp[:, 255:127:-1]
    em = em.reshape(NCORES, BS, 2, EMCOLS, 128).transpose(0, 2, 1, 3, 4)
    em = em.reshape(NCORES, 64, EMCOLS, 128)
    em = np.ascontiguousarray(em).astype(ml_dtypes.bfloat16)

    hc = hc.reshape(NCORES, BS, 1)
    return yt, em, hc


# ---------------------------------------------------------------- bass build

def build_nc():
    import concourse.bass as bass
    import concourse.tile as tile
    from concourse import bacc, mybir
    from concourse.ap import AP

    f32 = mybir.dt.float32
    bf16 = mybir.dt.bfloat16
    fp8 = mybir.dt.float8e4

    nc = bacc.Bacc(None, target_bir_lowering=False)

    yt_d = nc.declare_dram_parameter(
        "yt", [NQ, 128, NPAIR, KCH, 2, TQW], fp8, isOutput=False
    )
    em_d = nc.declare_dram_parameter("em", [64, EMCOLS, 128], bf16, isOutput=False)
    hc_d = nc.declare_dram_parameter("hc", [BS, 1], f32, isOutput=False)
    out_d = nc.declare_dram_parameter("out", [BS, 1], f32, isOutput=True)

    def rev_ap(ap, width):
        """Reverse the innermost (free) axis of a [P, width] AP view."""
        return AP(
            tensor=ap.tensor,
            offset=ap.offset + width - 1,
            ap=[list(ap.ap[0]), [-1, width]],
        )

    with tile.TileContext(nc) as tc:
        with ExitStack() as ctx:
            yp = ctx.enter_context(tc.tile_pool(name="yp", bufs=3))
            psp = ctx.enter_context(
                tc.tile_pool(name="psp", bufs=4, space=bass.MemorySpace.PSUM)
            )
            prp = ctx.enter_context(
                tc.tile_pool(name="prp", bufs=1, space=bass.MemorySpace.PSUM)
            )
            emp = ctx.enter_context(tc.tile_pool(name="emp", bufs=1))
            alp = ctx.enter_context(tc.tile_pool(name="alp", bufs=1))
            fin = ctx.enter_context(tc.tile_pool(name="fin", bufs=1))

            # emission tile: first cols land first so the DP starts early
            EM = emp.tile([64, EMCOLS, 128], bf16, name="em")
            nc.sync.dma_start(EM[:, 0:2], em_d[:, 0:2])
            nc.sync.dma_start(EM[:, 2:8], em_d[:, 2:8])
            nc.sync.dma_start(EM[:, 8:40], em_d[:, 8:40])
            nc.sync.dma_start(EM[:, 40:EMCOLS], em_d[:, 40:EMCOLS])

            HC = fin.tile([BS, 1], f32, name="hc")
            nc.sync.dma_start(HC[:], hc_d[:])

            ONES8 = fin.tile([128, 1], fp8, name="ones8")
            nc.vector.memset(ONES8[:], 1.0)
            ONESF = fin.tile([128, 1], f32, name="onesf")
            nc.vector.memset(ONESF[:], 1.0)
            # t-sum accumulator: samples 0:16 at partitions 0:16, samples
            # 16:32 at partitions 32:48 (PE out base must be 0/32/64)
            PRW = prp.tile([48, 1], f32, name="prw")

            # persistent DP state; o stored ONCE with zero pads both sides,
            # [(0,o)|(o,0)] read via an overlapping 2-seg AP (seg stride 1):
            # col0 pad | o@1(64) | pad65 | E0@66(65) | pad131 | E1@132(65) |
            # pad197 | q@198(64, col262 scratch)
            OC, E0, E1, Q = 1, 66, 132, 198
            MW = alp.tile([64, 340], bf16, name="mw")
            XX = alp.tile([64, 2 * L], bf16, name="xx")
            NRM = fin.tile([64, NN], f32, name="nrm")
            SQN = fin.tile([64, NN], f32, name="sqn")
            TMPR = alp.tile([64, 1], f32, name="tmpr")

            def seg2(tile_, off1, off2, width, rows=slice(None)):
                d = off2 - off1
                return tile_[rows, off1 : off1 + 2 * d].rearrange(
                    "p (a b) -> p a b", a=2, b=d
                )[:, :, 0:width]

            def o2seg(rows0, nrows):
                """[(0,o) | (o,0)] : overlapping 2-seg view, seg stride 1."""
                base = MW[slice(rows0, rows0 + nrows), 0:1]
                return AP(
                    tensor=base.tensor,
                    offset=base.offset,
                    ap=[list(base.ap[0]), [1, 2], [1, L + 1]],
                )

            nc.vector.memset(MW[:], 0.0)
            nc.vector.memset(XX[:], 0.0)
            nc.vector.memset(MW[:, E0 : E0 + 1], 1.0)  # e(0)=[1,0..] both chains
            # o(0)[0] = ul-lane 0 of em col 0 (fwd: t=0 ; bwd: t=255)
            nc.vector.tensor_copy(MW[:, OC : OC + 1], EM[:, 0, L : L + 1])

            # ---------------- rowsum path (PE + ACT, independent of DVE DP)
            lnq = {}
            for q in range(NQ):
                lnq[q] = fin.tile([128, NPAIR], f32, tag=f"lnq{q}", name=f"lnq{q}")
                for g in range(2):
                    yq = yp.tile(
                        [128, 8, KCH, 2, TQW], fp8, tag="yq", name="yq"
                    )
                    nc.sync.dma_start(yq[:], yt_d[q, :, 8 * g : 8 * g + 8])
                    ps = psp.tile([128, 8], f32, tag="ps", name="ps")
                    for pi in range(8):
                        for m in range(KCH):
                            nc.tensor.matmul(
                                ps[:, pi : pi + 1],
                                yq[:, pi, m],
                                ONES8[:],
                                start=(m == 0),
                                stop=(m == KCH - 1),
                            )
                    nc.scalar.activation(
                        lnq[q][:, 8 * g : 8 * g + 8], ps[:],
                        mybir.ActivationFunctionType.Ln, scale=LNSC,
                    )
                # per-sample t-sums: top partitions = samples 0..15,
                # bottom = samples 16..31, accumulated across quarters
                nc.tensor.matmul(
                    PRW[0:NPAIR], lnq[q][0:TQW, :], ONESF[0:TQW, :],
                    start=(q == 0), stop=(q == NQ - 1), skip_group_check=True,
                )
                nc.tensor.matmul(
                    PRW[32 : 32 + NPAIR], lnq[q][TQW:128, :], ONESF[TQW:128, :],
                    start=(q == 0), stop=(q == NQ - 1), skip_group_check=True,
                )

            # ---------------- merged DP (DVE only): 4 ops per step, ordered
            # [I1, x2, x1, I3] so x2 (independent) separates I1->x1; only
            # I3 and the next I1 are distance-1 dependent.
            def dp_step(k, rows0=0, nrows=64, emcol=None):
                rows = slice(rows0, rows0 + nrows)
                ecur = E0 if (k - 1) % 2 == 0 else E1
                enew = E1 if ecur == E0 else E0
                tt = k if emcol is None else emcol
                # I1: [E_new | q] = bcast(E_cur) + [(0,o) | (o,0)]
                nc.vector.tensor_add(
                    seg2(MW, enew, Q, L + 1, rows),
                    MW[rows, ecur : ecur + L + 1][:, None, :].broadcast_to(
                        [nrows, 2, L + 1]
                    ),
                    o2seg(rows0, nrows),
                )
                # x2 = (0,o) * su   (independent of I1)
                nc.vector.tensor_mul(
                    XX[rows, 0:L], MW[rows, 0:L], EM[rows, tt, 0:L]
                )
                # x1 = q * ul
                nc.vector.tensor_mul(
                    XX[rows, L : 2 * L], MW[rows, Q : Q + L],
                    EM[rows, tt, L : 2 * L],
                )
                # I3: o = x1 + x2
                nc.vector.tensor_add(
                    MW[rows, OC : OC + L], XX[rows, L : 2 * L], XX[rows, 0:L]
                )
                return enew

            PRS = fin.tile([48, 1], f32, name="prs")
            for k in range(1, 128):
                enew = dp_step(k)
                if k == 100:
                    # PRW complete long ago; slot the join into DP bubbles
                    nc.vector.tensor_copy(PRS[0:NPAIR, :], PRW[0:NPAIR])
                    nc.vector.tensor_copy(PRS[32:48, :], PRW[32:48])
                    nc.sync.dma_start(PRS[NPAIR:BS, :], PRS[32:48, :])
                if k in RENORMS:
                    r = RENORMS.index(k)
                    nc.vector.tensor_reduce(
                        NRM[:, r : r + 1], seg2(MW, OC, enew, L + 1),
                        mybir.AxisListType.XY, mybir.AluOpType.max,
                    )
                    nc.vector.reciprocal(TMPR[:], NRM[:, r : r + 1])
                    nc.vector.tensor_scalar_mul(
                        seg2(MW, OC, enew, L + 1),
                        seg2(MW, OC, enew, L + 1),
                        TMPR[:],
                    )
                    if r == NN - 2:
                        # ACT is idle mid-DP: take the Sqrt table load early
                        nc.scalar.activation(
                            SQN[:, 0 : NN - 1], NRM[:, 0 : NN - 1],
                            mybir.ActivationFunctionType.Sqrt,
                            scale=float(2.0 ** -32),
                        )

            # bwd state final: fetch psi rows + renorm logs BEFORE the fake
            # step so the tail latency overlaps it
            nc.scalar.activation(
                SQN[:, NN - 1 : NN], NRM[:, NN - 1 : NN],
                mybir.ActivationFunctionType.Sqrt, scale=float(2.0 ** -32),
            )
            scr_n = fin.tile([64, NN], f32, name="scrn")
            acc_n = fin.tile([64, 1], f32, name="accn")
            nc.scalar.activation(
                scr_n[:], SQN[:], mybir.ActivationFunctionType.Ln,
                accum_out=acc_n[:],
            )
            # one 2-seg DMA fetches [o+pad | E1] of the psi rows:
            # SCR cols 0:65 = (psio, pad), cols 65:130 = psiE
            SCR = fin.tile([BS, 130], bf16, name="scr")
            ACCB = fin.tile([BS, 1], f32, name="accb")
            nc.sync.dma_start(
                SCR[:, 0:130].rearrange("p (a b) -> p a b", a=2, b=65),
                seg2(MW, OC, E1, 65, slice(BS, 64)),
            )
            nc.sync.dma_start(ACCB[:], acc_n[BS:64, :])

            # fake fwd step (rows 0:32 only): em col 128 = [skip | 1]
            dp_step(128, rows0=0, nrows=BS, emcol=128)
            # after this: E0 rows 0:32 = a~e ; o rows 0:32 = a~o ;
            # E1 rows 32:64 = psiE_128 ; o rows 32:64 = psio_128 (in SCR)

            M1 = fin.tile([BS, L + 1], f32, name="m1")
            M2 = fin.tile([BS, L], f32, name="m2")
            R1 = fin.tile([BS, 1], f32, name="r1")
            LS = fin.tile([BS, 1], f32, name="ls")
            nc.vector.tensor_mul(
                M1[:], MW[0:BS, E0 : E0 + L + 1], rev_ap(SCR[:, 65:130], 65)
            )
            nc.vector.tensor_mul(
                M2[:], MW[0:BS, OC : OC + L], rev_ap(SCR[:, 0:64], L)
            )
            nc.vector.tensor_reduce(
                R1[:], M1[:], mybir.AxisListType.X, mybir.AluOpType.add
            )
            nc.vector.tensor_reduce(
                LS[:], M2[:], mybir.AxisListType.X, mybir.AluOpType.add
            )
            nc.vector.tensor_add(LS[:], LS[:], R1[:])
            ln_ls = fin.tile([BS, 1], f32, name="lnls")
            nc.scalar.activation(ln_ls[:], LS[:], mybir.ActivationFunctionType.Ln)

            # loss = PR - hc - 2*(acc_n_f + acc_n_b) - ln_ls + (16T-64NN)*ln2
            loss = fin.tile([BS, 1], f32, name="loss")
            t1 = fin.tile([BS, 1], f32, name="t1")
            nc.vector.tensor_add(t1[:], acc_n[0:BS, :], ACCB[:])
            nc.vector.tensor_add(t1[:], t1[:], t1[:])
            nc.vector.tensor_sub(loss[:], PRS[0:BS, :], t1[:])
            nc.vector.tensor_sub(loss[:], loss[:], HC[:])
            nc.vector.tensor_sub(loss[:], loss[:], ln_ls[:])
            nc.vector.tensor_single_scalar(
                loss[:], loss[:],
                float((16.0 * T - 64.0 * NN) * math.log(2.0)),
                mybir.AluOpType.add,
            )
            nc.sync.dma_start(out_d[:], loss[:])

    nc.compile()
    return nc


_NC_CACHE = {}


def _get_nc():
    if "nc" not in _NC_CACHE:
        _NC_CACHE["nc"] = build_nc()
    return _NC_CACHE["nc"]


# ---------------------------------------------------------------- entrypoint

def kernel(y_true: np.ndarray, y_pred: np.ndarray, _trace: bool = False):
    from concourse.bass_utils import run_bass_kernel_spmd

    yt, em, hc = host_prep(y_pred, y_true)

    in_maps = []
    for i in range(NCORES):
        in_maps.append({"yt": yt[i], "em": em[i], "hc": hc[i]})

    nc = _get_nc()
    res = run_bass_kernel_spmd(nc, in_maps, list(range(NCORES)), trace=_trace)
    out = np.concatenate([res.results[i]["out"] for i in range(NCORES)], axis=0)
    if _trace:
        return out.astype(np.float32), res
    return out.astype(np.float32)
